# revision 29
# baseline (speedup 1.0000x reference)
"""Multihead attention (B=2, S=2048, D=1024, H=16) on 8 TRN2 NeuronCores.

Sharding: core c -> batch b = c//4, head-group g = c%4 (4 heads, 256
features). Each core computes q/k/v projections for its 256 features,
attention for its 4 heads, and a row-parallel partial of the output
projection. Host sums the 4 partials per batch and transposes back.

Key compaction: masked keys contribute exactly 0 (exp underflow), so the
host gathers only unmasked keys/values (M_b ~ 1024 of 2048) padded to MT
tiles of 128; pad keys get the -9e9 bias.

Two hardware behaviors shape the schedule:
 - The PE clock ramps with sustained use (~0.65 -> 2.4 GHz over ~6us of
   continuous execution) and sags on idle gaps, so every phase keeps the
   PE stream dense and handoffs avoid gating the PE on VectorE.
 - 64-row matmuls placed on PE row-tiles T0 (SBUF partitions 0:64) and T8
   (64:128) execute concurrently when alternated (measured 2.8x), but a
   mode switch (128-row matmul) drains the array. So the whole attention
   phase is 64-contraction: scores per head land on T0/T8 by head parity,
   and the AV contraction is split into lo/hi key halves accumulated
   separately (combined by one DVE add before the softmax divide).

Attention runs in segments s = (i-half, head-pair): scores+exp of segment
s (ScalarE-paced, tags SA/SB) overlap the AV phase of segment s-1 (po on
T4..T7, n-chunk serial). vproj fills the PE during the first window; the
output projection fills the last. exp(scale*x + maskbias) is fused on
ScalarE; the divide is DVE reciprocal -> GpSimd partition broadcast ->
DVE multiply straight into the ot tiles.
"""

import math

import numpy as np

B, S, D, H = 2, 2048, 1024, 16
NCORES = 8
GH = 4                  # heads per core
HD = D // H             # 64
F = GH * HD             # 256 local features
SCALE = 1.0 / math.sqrt(HD)
NEG = np.float32(-9e9)

KT = D // 128           # 8 contraction tiles (projections)
FT = F // 128           # 2 local-feature tiles
DT = D // 128           # 8 output-feature tiles

TRACE = False
LAST_EXEC_NS = None
LAST_RESULTS = None

_STATE = {}


def _build(MT):
    import concourse.bacc as bacc
    import concourse.mybir as mybir
    from concourse.tile import TileContext

    f32 = mybir.dt.float32
    bf16 = mybir.dt.bfloat16
    Exp = mybir.ActivationFunctionType.Exp
    MP = MT * 128

    nc = bacc.Bacc("TRN2", target_bir_lowering=False, debug=False,
                   num_devices=NCORES)

    xq_d = nc.declare_dram_parameter("xqT", [D, S], bf16, isOutput=False)
    xk_d = nc.declare_dram_parameter("xkT", [D, MP], bf16, isOutput=False)
    xv_d = nc.declare_dram_parameter("xv3", [MT, 128, D], bf16, isOutput=False)
    wq_d = nc.declare_dram_parameter("wqT", [D, F], bf16, isOutput=False)
    wk_d = nc.declare_dram_parameter("wkT", [D, F], bf16, isOutput=False)
    wv_d = nc.declare_dram_parameter("wvT", [D, F], bf16, isOutput=False)
    wo_d = nc.declare_dram_parameter("woT", [F, D], bf16, isOutput=False)
    bq_d = nc.declare_dram_parameter("bq2", [128, FT], f32, isOutput=False)
    bk_d = nc.declare_dram_parameter("bk2", [128, FT], f32, isOutput=False)
    bv_d = nc.declare_dram_parameter("bvb", [128, F], f32, isOutput=False)
    bo_d = nc.declare_dram_parameter("bo2", [128, DT], f32, isOutput=False)
    mk_d = nc.declare_dram_parameter("mask2", [128, MT], f32, isOutput=False)
    out_d = nc.declare_dram_parameter("outT", [D, S], bf16, isOutput=True)

    kchunks = []
    c0 = 0
    while c0 < MP:
        w = min(512, MP - c0)
        kchunks.append((c0, w))
        c0 += w

    with TileContext(nc) as tc:
        with tc.tile_pool(name="persist", bufs=1) as pp, \
             tc.tile_pool(name="expp", bufs=32) as ep, \
             tc.tile_pool(name="divp", bufs=2) as dp, \
             tc.tile_pool(name="cmbp", bufs=2) as cp, \
             tc.tile_pool(name="ostage", bufs=6) as osp, \
             tc.tile_pool(name="small", bufs=2) as sp, \
             tc.tile_pool(name="xkp", bufs=8) as xkp, \
             tc.tile_pool(name="xvp", bufs=4) as xvp, \
             tc.tile_pool(name="xqp", bufs=8) as xqp:

            def ptile(shape, dtype, name):
                return pp.tile(shape, dtype, name=name, tag=name)

            # ---- persistent SBUF tensors ----
            wq_sb = [ptile([128, F], bf16, f"wq{k}") for k in range(KT)]
            wk_sb = [ptile([128, F], bf16, f"wk{k}") for k in range(KT)]
            wv_sb = [ptile([128, F], bf16, f"wv{k}") for k in range(KT)]
            wo_sb = [ptile([128, D], bf16, f"wo{t}") for t in range(FT)]
            bqt = ptile([128, FT], f32, "bqt")
            bkt = ptile([128, FT], f32, "bkt")
            bot = ptile([128, DT], f32, "bot")
            mkt = ptile([128, MT], f32, "mkt")
            bvb = ptile([128, F], f32, "bvb")
            qT_sb = [ptile([128, S], bf16, f"qT{t}") for t in range(FT)]
            kT_sb = [ptile([128, MP], bf16, f"kT{t}") for t in range(FT)]
            va_sb = [ptile([128, GH * 128], bf16, f"va{j}") for j in range(MT)]
            # ot split per i-half so out_proj for half 0 carries no (false)
            # dependency on half 1's divides
            ot_sb = [[ptile([128, 1024], bf16, f"ot{t}h{hf}") for hf in range(2)]
                     for t in range(FT)]

            for j in range(MT):
                nc.vector.memset(va_sb[j][:], 0.0)
                for h in range(GH):
                    nc.vector.memset(va_sb[j][:, h * 128:h * 128 + 1], 1.0)

            # DMAs in consumption order; big tiles split per 64 partitions
            # so the first tiles land fast across parallel queues.
            nc.sync.dma_start(out=mkt[:], in_=mk_d[:])
            nc.sync.dma_start(out=bkt[:], in_=bk_d[:])
            nc.sync.dma_start(out=bqt[:], in_=bq_d[:])
            nc.sync.dma_start(out=bvb[:], in_=bv_d[:])
            nc.sync.dma_start(out=bot[:], in_=bo_d[:])

            # split big input tiles by partitions (keeps full-width DMA
            # lines) so each tile rides two queues in parallel
            def dma2(dst, src):
                nc.sync.dma_start(out=dst[0:64, :], in_=src[0:64, :])
                nc.sync.dma_start(out=dst[64:128, :], in_=src[64:128, :])

            xk_sb, xv_sb, xq_sb = [], [], []
            for k in range(KT):
                nc.sync.dma_start(out=wk_sb[k][:],
                                  in_=wk_d[k * 128:(k + 1) * 128, :])
                xt = xkp.tile([128, MP], bf16, name=f"xk{k}", tag="xk")
                dma2(xt, xk_d[k * 128:(k + 1) * 128, :])
                xk_sb.append(xt)
            for k in range(KT):
                nc.sync.dma_start(out=wq_sb[k][:],
                                  in_=wq_d[k * 128:(k + 1) * 128, :])
                xt = xqp.tile([128, S], bf16, name=f"xq{k}", tag="xq")
                dma2(xt, xq_d[k * 128:(k + 1) * 128, :])
                xq_sb.append(xt)
            for k in range(KT):
                nc.sync.dma_start(out=wv_sb[k][:],
                                  in_=wv_d[k * 128:(k + 1) * 128, :])
            for st in range(MT):
                xt = xvp.tile([128, D], bf16, name=f"xv{st}", tag="xv")
                nc.sync.dma_start(out=xt[:], in_=xv_d[st])
                xv_sb.append(xt)
            for t in range(FT):
                nc.sync.dma_start(out=wo_sb[t][:],
                                  in_=wo_d[t * 128:(t + 1) * 128, :])

            with tc.tile_pool(name="psB", bufs=1, space="PSUM") as psB:

                def ps2(tag):  # 2-bank [128,1024] tile
                    return psB.tile([128, 1024], mybir.dt.float32,
                                    name=tag, tag=tag)

                def ps1(tag):  # 1-bank [128,512] tile
                    return psB.tile([128, 512], mybir.dt.float32,
                                    name=tag, tag=tag)

                # ---- kproj: groups (t, chunk) on SA, SB, T4, T5 ----
                kg = [(t, ci) for t in range(FT) for ci in range(len(kchunks))]
                accs = []
                tile = None
                for gi, (t, ci) in enumerate(kg):
                    w = kchunks[ci][1]
                    if gi < 4:
                        if gi % 2 == 0:
                            tile = ps2("SA" if gi == 0 else "SB")
                        accs.append(tile[:, (gi % 2) * 512:(gi % 2) * 512 + w])
                    else:
                        accs.append(ps1("T4" if gi % 2 == 0 else "T5")[:, 0:w])
                for k in range(KT):
                    for gi, (t, ci) in enumerate(kg):
                        c0, w = kchunks[ci]
                        nc.tensor.matmul(
                            accs[gi],
                            lhsT=wk_sb[k][:, t * 128:(t + 1) * 128],
                            rhs=xk_sb[k][:, c0:c0 + w],
                            start=(k == 0), stop=(k == KT - 1))
                for gi, (t, ci) in enumerate(kg):
                    c0, w = kchunks[ci]
                    nc.vector.tensor_scalar_add(
                        kT_sb[t][:, c0:c0 + w], accs[gi], bkt[:, t:t + 1])

                # ---- qproj: the two groups attention touches first run now
                # (on T6/T7, untouched by kproj, so no PE wait on kproj's
                # bias copies); the remaining 6 groups are emitted inside
                # the A_0 window as PE filler (T4/T5 pairs, xq re-read from
                # SBUF) ----
                def qproj_pair(groups, accs):
                    for k in range(KT):
                        for gi, (t, ch) in enumerate(groups):
                            nc.tensor.matmul(
                                accs[gi],
                                lhsT=wq_sb[k][:, t * 128:(t + 1) * 128],
                                rhs=xq_sb[k][:, ch * 512:(ch + 1) * 512],
                                start=(k == 0), stop=(k == KT - 1))
                    for gi, (t, ch) in enumerate(groups):
                        nc.vector.tensor_scalar_add(
                            qT_sb[t][:, ch * 512:(ch + 1) * 512],
                            accs[gi], bqt[:, t:t + 1])

                qproj_pair([(0, 0), (0, 1)], [ps1("T6")[:, :], ps1("T7")[:, :]])
                qrest = [[(1, 0), (1, 1)], [(0, 2), (0, 3)], [(1, 2), (1, 3)]]

                # ---- attention: segments s = (half, pair) ----
                SEGS = [(h, p) for h in range(2) for p in range(FT)]
                e_tiles = {}

                def emit_scores(s, j):
                    half, pair = SEGS[s]
                    i0 = half * 1024
                    sa = ps2("SA")
                    sb = ps2("SB")
                    for n in range(2):
                        for off, stile in ((0, sa), (HD, sb)):
                            nc.tensor.matmul(
                                stile[:, n * 512:(n + 1) * 512],
                                lhsT=kT_sb[pair][off:off + HD,
                                                 j * 128:(j + 1) * 128],
                                rhs=qT_sb[pair][off:off + HD,
                                                i0 + n * 512:i0 + (n + 1) * 512],
                                start=True, stop=True)
                    for hp, stile in ((0, sa), (1, sb)):
                        e = ep.tile([128, 1024], bf16, name="e", tag="e")
                        nc.scalar.activation(e[:], stile[:], Exp,
                                             bias=mkt[:, j:j + 1], scale=SCALE)
                        e_tiles[(s, hp, j)] = e

                bstate = {}
                PO_TAGS = {(0, 0): "T4", (0, 1): "T5", (1, 0): "T6", (1, 1): "T7"}

                def emit_av(s, w):
                    # B-phase substep w: n = w//MT, j = w%MT; AV contraction
                    # split into lo (T0) / hi (T8) key halves per head.
                    half, pair = SEGS[s]
                    n, j = divmod(w, MT)
                    if j == 0:
                        for hp in range(2):
                            for lh in range(2):
                                bstate[(s, n, hp, lh)] = ps1(PO_TAGS[(hp, lh)])
                    for hp in range(2):
                        h = 2 * pair + hp
                        for lh in range(2):
                            po = bstate[(s, n, hp, lh)]
                            b0 = lh * HD
                            nc.tensor.matmul(
                                po[:],
                                lhsT=va_sb[j][b0:b0 + HD,
                                              h * 128:(h + 1) * 128],
                                rhs=e_tiles[(s, hp, j)][b0:b0 + HD,
                                                        n * 512:(n + 1) * 512],
                                start=(j == 0), stop=(j == MT - 1))
                    if j == MT - 1:
                        i0 = half * 1024 + n * 512
                        cmbs, recs = [], []
                        for hp in range(2):
                            lo = bstate.pop((s, n, hp, 0))
                            hi = bstate.pop((s, n, hp, 1))
                            # DVE cannot read two PSUM operands in one op:
                            # stage lo in SBUF, then add hi (PSUM) to it.
                            los = cp.tile([128, 512], f32, name="los",
                                          tag="los")
                            nc.vector.tensor_copy(los[:], lo[:])
                            cmb = cp.tile([128, 512], f32, name="cmb",
                                          tag="cmb")
                            nc.vector.tensor_tensor(
                                out=cmb[:], in0=hi[:], in1=los[:],
                                op=mybir.AluOpType.add)
                            rec = sp.tile([1, 512], f32, name="rec", tag="rec")
                            nc.vector.reciprocal_approx_fast(
                                out=rec[:], in_=cmb[0:1, :])
                            recb = dp.tile([128, 512], f32,
                                           name="recb", tag="recb")
                            nc.gpsimd.partition_broadcast(recb[:], rec[:])
                            cmbs.append(cmb)
                            recs.append(recb)
                        for hp in range(2):
                            nc.vector.tensor_tensor(
                                out=ot_sb[pair][half][hp * HD:hp * HD + HD,
                                                      n * 512:(n + 1) * 512],
                                in0=cmbs[hp][HD:128, :],
                                in1=recs[hp][HD:128, :],
                                op=mybir.AluOpType.mult)

                def emit_vproj(st):
                    # 64-contraction lo/hi halves on T0/T8 so the A_0 window
                    # stays in one PE tiling mode; combined during bias add.
                    pv_lo = ps1("T6")
                    pv_hi = ps1("T7")
                    for k in range(KT):
                        for b0, pv in ((0, pv_lo), (HD, pv_hi)):
                            nc.tensor.matmul(
                                pv[:, 0:F],
                                lhsT=xv_sb[st][b0:b0 + HD,
                                               k * 128:(k + 1) * 128],
                                rhs=wv_sb[k][b0:b0 + HD, :],
                                start=(k == 0), stop=(k == KT - 1))
                    lvs = cp.tile([128, F], f32, name="lvs", tag="lvs")
                    nc.vector.tensor_copy(lvs[:], pv_lo[:, 0:F])
                    cv = cp.tile([128, F], f32, name="cmbv", tag="cmbv")
                    nc.vector.tensor_tensor(
                        out=cv[:], in0=pv_hi[:, 0:F], in1=lvs[:],
                        op=mybir.AluOpType.add)
                    for h in range(GH):
                        nc.vector.tensor_tensor(
                            out=va_sb[st][:, h * 128 + HD:(h + 1) * 128],
                            in0=cv[:, h * HD:(h + 1) * HD],
                            in1=bvb[:, h * HD:(h + 1) * HD],
                            op=mybir.AluOpType.add)

                def outproj_job(ih, do, n, acc, use_scalar):
                    # one [128,512] i-chunk of one do-tile; bias adds
                    # alternate VectorE/ScalarE across jobs
                    i0 = ih * 1024 + n * 512
                    for t in range(FT):
                        nc.tensor.matmul(
                            acc,
                            lhsT=wo_sb[t][:, do * 128:(do + 1) * 128],
                            rhs=ot_sb[t][ih][:, n * 512:(n + 1) * 512],
                            start=(t == 0), stop=(t == FT - 1))
                    stg = osp.tile([128, 512], bf16, name="stg", tag="stg")
                    if use_scalar:
                        nc.scalar.add(stg[:], acc, bot[:, do:do + 1])
                    else:
                        nc.vector.tensor_scalar_add(
                            stg[:], acc, bot[:, do:do + 1])
                    for i in range(2):
                        nc.sync.dma_start(
                            out=out_d[do * 128 + i * 64:
                                      do * 128 + (i + 1) * 64,
                                      i0:i0 + 512],
                            in_=stg[i * 64:(i + 1) * 64, :])

                def slot_gen(extra):
                    # lazy PSUM slot rotation: SA/SB halves (+T4..T7 once
                    # the AV accumulators have drained)
                    while True:
                        tA = ps2("SA")
                        yield tA[:, 0:512]
                        yield tA[:, 512:1024]
                        tB = ps2("SB")
                        yield tB[:, 0:512]
                        yield tB[:, 512:1024]
                        if extra:
                            for t in ("T4", "T5", "T6", "T7"):
                                yield ps1(t)[:, :]

                def run_outproj(jobs, slots):
                    for ji, (ih, do, n) in enumerate(jobs):
                        outproj_job(ih, do, n, next(slots), ji % 2 == 1)

                NW = 2 * MT

                # A_0 window: scores for seg 0, vproj fill on odd steps,
                # and the remaining qproj groups as extra PE filler
                for w in range(NW):
                    if w % 2 == 0:
                        emit_scores(0, w // 2)
                    elif w // 2 < MT:
                        emit_vproj(w // 2)
                    if w in (2, 8, 14):
                        sub = qrest[(w - 2) // 6]
                        qproj_pair(sub, [ps1("T4")[:, :], ps1("T5")[:, :]])

                # [A_s+1 || B_s] windows; at window start the scores go
                # first (nothing pending on them) so ScalarE restarts early
                for s in range(len(SEGS) - 1):
                    for w in range(NW):
                        if w == 0:
                            emit_scores(s + 1, 0)
                            emit_av(s, w)
                        else:
                            emit_av(s, w)
                            if w % 2 == 0:
                                emit_scores(s + 1, w // 2)

                # B_3 window, phase-ordered to keep the PE dense: AV n0
                # block; all i-half-0 jobs (independent of B_3); AV n1
                # block; then i-half-1 jobs split by n so only the last 8
                # depend on the final divides.
                s3 = len(SEGS) - 1
                for w in range(MT):
                    emit_av(s3, w)
                slots = slot_gen(False)
                run_outproj([(0, do, n) for do in range(DT)
                             for n in range(2)], slots)
                for w in range(MT, NW):
                    emit_av(s3, w)
                run_outproj([(1, do, 0) for do in range(DT)], slot_gen(False))
                run_outproj([(1, do, 1) for do in range(DT)], slot_gen(True))

    nc.compile()
    return nc


def kernel(query, key, value, src_mask, Wq, bq, Wk, bk, Wv, bv, Wo, bo, nhead):
    global LAST_EXEC_NS, LAST_RESULTS
    import ml_dtypes
    from concourse.bass_utils import run_bass_kernel_spmd

    assert int(nhead) == H
    bf16 = ml_dtypes.bfloat16
    query = np.asarray(query, dtype=np.float32)
    key = np.asarray(key, dtype=np.float32)
    value = np.asarray(value, dtype=np.float32)
    src_mask = np.asarray(src_mask)
    Wq, bq = np.asarray(Wq, np.float32), np.asarray(bq, np.float32)
    Wk, bk = np.asarray(Wk, np.float32), np.asarray(bk, np.float32)
    Wv, bv = np.asarray(Wv, np.float32), np.asarray(bv, np.float32)
    Wo, bo = np.asarray(Wo, np.float32), np.asarray(bo, np.float32)

    # host-side key/value compaction
    idxs = [np.nonzero(~src_mask[b])[0] for b in range(B)]
    Ms = [len(ix) for ix in idxs]
    MT = max(2, (max(Ms) + 127) // 128)
    MP = MT * 128

    if ("nc", MT) not in _STATE:
        _STATE[("nc", MT)] = _build(MT)
    nc = _STATE[("nc", MT)]

    xqT, xkT, xvT, maskf = [], [], [], []
    for b in range(B):
        xqT.append(np.ascontiguousarray(query[b].T).astype(bf16))
        kc = np.zeros((MP, D), np.float32)
        kc[:Ms[b]] = key[b][idxs[b]]
        vc = np.zeros((MP, D), np.float32)
        vc[:Ms[b]] = value[b][idxs[b]]
        xkT.append(np.ascontiguousarray(kc.T).astype(bf16))
        xvT.append(np.ascontiguousarray(
            vc.T.reshape(KT, 128, MT, 128).transpose(2, 1, 0, 3)
            .reshape(MT, 128, D)).astype(bf16))
        mk = np.where(np.arange(MP) < Ms[b], np.float32(0), NEG)
        maskf.append(np.ascontiguousarray(mk.reshape(MT, 128).T))

    wqT, wkT, wvT, woT, bqs, bks, bvs = [], [], [], [], [], [], []
    for g in range(NCORES // B):
        gs, ge = g * F, (g + 1) * F
        wqT.append(np.ascontiguousarray(Wq[gs:ge, :].T).astype(bf16))
        wkT.append(np.ascontiguousarray(Wk[gs:ge, :].T).astype(bf16))
        wvT.append(np.ascontiguousarray(Wv[gs:ge, :].T).astype(bf16))
        woT.append(np.ascontiguousarray(Wo[:, gs:ge].T).astype(bf16))
        bqs.append(np.ascontiguousarray(bq[gs:ge].reshape(FT, 128).T))
        bks.append(np.ascontiguousarray(bk[gs:ge].reshape(FT, 128).T))
        bvs.append(np.ascontiguousarray(
            np.tile(bv[gs:ge][None, :], (128, 1)).astype(np.float32)))
    bo2 = np.ascontiguousarray(bo.reshape(DT, 128).T)
    bo_zero = np.zeros_like(bo2)

    in_maps = []
    for c in range(NCORES):
        b, g = c // (NCORES // B), c % (NCORES // B)
        in_maps.append({
            "xqT": xqT[b], "xkT": xkT[b], "xv3": xvT[b],
            "wqT": wqT[g], "wkT": wkT[g], "wvT": wvT[g], "woT": woT[g],
            "bq2": bqs[g], "bk2": bks[g], "bvb": bvs[g],
            "bo2": bo2 if g == 0 else bo_zero,
            "mask2": maskf[b],
        })

    kwargs = {}
    if TRACE:
        kwargs = dict(trace=True)
    res = run_bass_kernel_spmd(nc, in_maps, core_ids=list(range(NCORES)),
                               **kwargs)
    LAST_EXEC_NS = res.exec_time_ns
    LAST_RESULTS = res

    out = np.empty((B, S, D), dtype=np.float32)
    for b in range(B):
        acc = res.results[b * (NCORES // B)]["outT"].astype(np.float32)
        for g in range(1, NCORES // B):
            acc = acc + res.results[b * (NCORES // B) + g]["outT"]
        out[b] = acc.T
    return out


# revision 33
# speedup vs baseline: 1.0641x; 1.0641x over previous
"""Multihead attention (B=2, S=2048, D=1024, H=16) on 8 TRN2 NeuronCores.

Sharding: core c -> batch b = c//4, head-group g = c%4 (4 heads, 256
features). Each core computes q/k/v projections for its 256 features,
attention for its 4 heads, and a row-parallel partial of the output
projection. Host sums the 4 partials per batch and transposes back.

Key compaction: masked keys contribute exactly 0 (exp underflow), so the
host gathers only unmasked keys/values (M_b ~ 1024 of 2048) padded to MT
tiles of 128; pad keys get the -9e9 bias.

Two hardware behaviors shape the schedule:
 - The PE clock ramps with sustained use (~0.65 -> 2.4 GHz over ~6us of
   continuous execution) and sags on idle gaps, so every phase keeps the
   PE stream dense and handoffs avoid gating the PE on VectorE.
 - 64-row matmuls placed on PE row-tiles T0 (SBUF partitions 0:64) and T8
   (64:128) execute concurrently when alternated (measured 2.8x), but a
   mode switch (128-row matmul) drains the array. So the whole attention
   phase is 64-contraction: scores per head land on T0/T8 by head parity,
   and the AV contraction is split into lo/hi key halves accumulated
   separately (combined by one DVE add before the softmax divide).

Attention runs in segments s = (i-half, head-pair): scores+exp of segment
s (ScalarE-paced, tags SA/SB) overlap the AV phase of segment s-1 (po on
T4..T7, n-chunk serial). vproj fills the PE during the first window; the
output projection fills the last. exp(scale*x + maskbias) is fused on
ScalarE; the divide is DVE reciprocal -> GpSimd partition broadcast ->
DVE multiply straight into the ot tiles.
"""

import math

import numpy as np

B, S, D, H = 2, 2048, 1024, 16
NCORES = 8
GH = 4                  # heads per core
HD = D // H             # 64
F = GH * HD             # 256 local features
SCALE = 1.0 / math.sqrt(HD)
NEG = np.float32(-9e9)

KT = D // 128           # 8 contraction tiles (projections)
FT = F // 128           # 2 local-feature tiles
DT = D // 128           # 8 output-feature tiles

TRACE = False
LAST_EXEC_NS = None
LAST_RESULTS = None

_STATE = {}


def _build(MT):
    import concourse.bacc as bacc
    import concourse.mybir as mybir
    from concourse.tile import TileContext

    f32 = mybir.dt.float32
    bf16 = mybir.dt.bfloat16
    Exp = mybir.ActivationFunctionType.Exp
    MP = MT * 128

    nc = bacc.Bacc("TRN2", target_bir_lowering=False, debug=False,
                   num_devices=NCORES)

    xq_d = nc.declare_dram_parameter("xqT", [D, S], bf16, isOutput=False)
    xk_d = nc.declare_dram_parameter("xkT", [D, MP], bf16, isOutput=False)
    xv_d = nc.declare_dram_parameter("xv3", [MT, 128, D], bf16, isOutput=False)
    wq_d = nc.declare_dram_parameter("wqT", [D, F], bf16, isOutput=False)
    wk_d = nc.declare_dram_parameter("wkT", [D, F], bf16, isOutput=False)
    wv_d = nc.declare_dram_parameter("wvT", [D, F], bf16, isOutput=False)
    wo_d = nc.declare_dram_parameter("woT", [F, D], bf16, isOutput=False)
    bq_d = nc.declare_dram_parameter("bq2", [128, FT], f32, isOutput=False)
    bk_d = nc.declare_dram_parameter("bk2", [128, FT], f32, isOutput=False)
    bv_d = nc.declare_dram_parameter("bvb", [128, F], f32, isOutput=False)
    bo_d = nc.declare_dram_parameter("bo2", [128, DT], f32, isOutput=False)
    mk_d = nc.declare_dram_parameter("mask2", [128, MT], f32, isOutput=False)
    out_d = nc.declare_dram_parameter("outT", [D, S], bf16, isOutput=True)

    kchunks = []
    c0 = 0
    while c0 < MP:
        w = min(512, MP - c0)
        kchunks.append((c0, w))
        c0 += w

    with TileContext(nc) as tc:
        with tc.tile_pool(name="persist", bufs=1) as pp, \
             tc.tile_pool(name="expp", bufs=32) as ep, \
             tc.tile_pool(name="divp", bufs=2) as dp, \
             tc.tile_pool(name="cmbp", bufs=2) as cp, \
             tc.tile_pool(name="ostage", bufs=6) as osp, \
             tc.tile_pool(name="small", bufs=2) as sp, \
             tc.tile_pool(name="xkp", bufs=8) as xkp, \
             tc.tile_pool(name="xvp", bufs=4) as xvp, \
             tc.tile_pool(name="xqp", bufs=8) as xqp:

            def ptile(shape, dtype, name):
                return pp.tile(shape, dtype, name=name, tag=name)

            # ---- persistent SBUF tensors ----
            wq_sb = [ptile([128, F], bf16, f"wq{k}") for k in range(KT)]
            wk_sb = [ptile([128, F], bf16, f"wk{k}") for k in range(KT)]
            wv_sb = [ptile([128, F], bf16, f"wv{k}") for k in range(KT)]
            wo_sb = [ptile([128, D], bf16, f"wo{t}") for t in range(FT)]
            bqt = ptile([128, FT], f32, "bqt")
            bkt = ptile([128, FT], f32, "bkt")
            bot = ptile([128, DT], f32, "bot")
            mkt = ptile([128, MT], f32, "mkt")
            bvb = ptile([128, F], f32, "bvb")
            qT_sb = [ptile([128, S], bf16, f"qT{t}") for t in range(FT)]
            kT_sb = [ptile([128, MP], bf16, f"kT{t}") for t in range(FT)]
            va_sb = [ptile([128, GH * 128], bf16, f"va{j}") for j in range(MT)]
            # ot split per i-half so out_proj for half 0 carries no (false)
            # dependency on half 1's divides
            ot_sb = [[ptile([128, 1024], bf16, f"ot{t}h{hf}") for hf in range(2)]
                     for t in range(FT)]

            for j in range(MT):
                nc.vector.memset(va_sb[j][:], 0.0)
                for h in range(GH):
                    nc.vector.memset(va_sb[j][:, h * 128:h * 128 + 1], 1.0)

            # DMAs in consumption order; big tiles split per 64 partitions
            # so the first tiles land fast across parallel queues.
            nc.sync.dma_start(out=mkt[:], in_=mk_d[:])
            nc.sync.dma_start(out=bkt[:], in_=bk_d[:])
            nc.sync.dma_start(out=bqt[:], in_=bq_d[:])
            nc.sync.dma_start(out=bvb[:], in_=bv_d[:])
            nc.sync.dma_start(out=bot[:], in_=bo_d[:])

            xk_sb, xv_sb, xq_sb = [], [], []
            for k in range(KT):
                nc.sync.dma_start(out=wk_sb[k][:],
                                  in_=wk_d[k * 128:(k + 1) * 128, :])
                xt = xkp.tile([128, MP], bf16, name=f"xk{k}", tag="xk")
                nc.sync.dma_start(out=xt[:], in_=xk_d[k * 128:(k + 1) * 128, :])
                xk_sb.append(xt)
            for k in range(KT):
                nc.sync.dma_start(out=wq_sb[k][:],
                                  in_=wq_d[k * 128:(k + 1) * 128, :])
                xt = xqp.tile([128, S], bf16, name=f"xq{k}", tag="xq")
                nc.sync.dma_start(out=xt[:], in_=xq_d[k * 128:(k + 1) * 128, :])
                xq_sb.append(xt)
            for k in range(KT):
                nc.sync.dma_start(out=wv_sb[k][:],
                                  in_=wv_d[k * 128:(k + 1) * 128, :])
            for st in range(MT):
                xt = xvp.tile([128, D], bf16, name=f"xv{st}", tag="xv")
                nc.sync.dma_start(out=xt[:], in_=xv_d[st])
                xv_sb.append(xt)
            for t in range(FT):
                nc.sync.dma_start(out=wo_sb[t][:],
                                  in_=wo_d[t * 128:(t + 1) * 128, :])

            with tc.tile_pool(name="psB", bufs=1, space="PSUM") as psB:

                def ps2(tag):  # 2-bank [128,1024] tile
                    return psB.tile([128, 1024], mybir.dt.float32,
                                    name=tag, tag=tag)

                def ps1(tag):  # 1-bank [128,512] tile
                    return psB.tile([128, 512], mybir.dt.float32,
                                    name=tag, tag=tag)

                # ---- kproj: groups (t, chunk) on SA, SB, T4, T5 ----
                kg = [(t, ci) for t in range(FT) for ci in range(len(kchunks))]
                accs = []
                tile = None
                for gi, (t, ci) in enumerate(kg):
                    w = kchunks[ci][1]
                    if gi < 4:
                        if gi % 2 == 0:
                            tile = ps2("SA" if gi == 0 else "SB")
                        accs.append(tile[:, (gi % 2) * 512:(gi % 2) * 512 + w])
                    else:
                        accs.append(ps1("T4" if gi % 2 == 0 else "T5")[:, 0:w])
                for k in range(KT):
                    for gi, (t, ci) in enumerate(kg):
                        c0, w = kchunks[ci]
                        nc.tensor.matmul(
                            accs[gi],
                            lhsT=wk_sb[k][:, t * 128:(t + 1) * 128],
                            rhs=xk_sb[k][:, c0:c0 + w],
                            start=(k == 0), stop=(k == KT - 1))
                for gi, (t, ci) in enumerate(kg):
                    c0, w = kchunks[ci]
                    nc.vector.tensor_scalar_add(
                        kT_sb[t][:, c0:c0 + w], accs[gi], bkt[:, t:t + 1])

                # ---- qproj: the two groups attention touches first run now
                # (on T6/T7, untouched by kproj, so no PE wait on kproj's
                # bias copies); the remaining 6 groups are emitted inside
                # the A_0 window as PE filler (T4/T5 pairs, xq re-read from
                # SBUF) ----
                def qproj_pair(groups, accs):
                    for k in range(KT):
                        for gi, (t, ch) in enumerate(groups):
                            nc.tensor.matmul(
                                accs[gi],
                                lhsT=wq_sb[k][:, t * 128:(t + 1) * 128],
                                rhs=xq_sb[k][:, ch * 512:(ch + 1) * 512],
                                start=(k == 0), stop=(k == KT - 1))
                    for gi, (t, ch) in enumerate(groups):
                        nc.vector.tensor_scalar_add(
                            qT_sb[t][:, ch * 512:(ch + 1) * 512],
                            accs[gi], bqt[:, t:t + 1])

                qproj_pair([(0, 0), (0, 1)], [ps1("T6")[:, :], ps1("T7")[:, :]])
                qrest = [[(1, 0), (1, 1)], [(0, 2), (0, 3)], [(1, 2), (1, 3)]]

                # ---- attention: segments s = (half, pair) ----
                SEGS = [(h, p) for h in range(2) for p in range(FT)]
                e_tiles = {}

                def emit_scores(s, j):
                    half, pair = SEGS[s]
                    i0 = half * 1024
                    sa = ps2("SA")
                    sb = ps2("SB")
                    for n in range(2):
                        for off, stile in ((0, sa), (HD, sb)):
                            nc.tensor.matmul(
                                stile[:, n * 512:(n + 1) * 512],
                                lhsT=kT_sb[pair][off:off + HD,
                                                 j * 128:(j + 1) * 128],
                                rhs=qT_sb[pair][off:off + HD,
                                                i0 + n * 512:i0 + (n + 1) * 512],
                                start=True, stop=True)
                    for hp, stile in ((0, sa), (1, sb)):
                        e = ep.tile([128, 1024], bf16, name="e", tag="e")
                        nc.scalar.activation(e[:], stile[:], Exp,
                                             bias=mkt[:, j:j + 1], scale=SCALE)
                        e_tiles[(s, hp, j)] = e

                bstate = {}
                PO_TAGS = {(0, 0): "T4", (0, 1): "T5", (1, 0): "T6", (1, 1): "T7"}

                def emit_av(s, w):
                    # B-phase substep w: n = w//MT, j = w%MT; AV contraction
                    # split into lo (T0) / hi (T8) key halves per head.
                    half, pair = SEGS[s]
                    n, j = divmod(w, MT)
                    if j == 0:
                        for hp in range(2):
                            for lh in range(2):
                                bstate[(s, n, hp, lh)] = ps1(PO_TAGS[(hp, lh)])
                    for hp in range(2):
                        h = 2 * pair + hp
                        for lh in range(2):
                            po = bstate[(s, n, hp, lh)]
                            b0 = lh * HD
                            nc.tensor.matmul(
                                po[:],
                                lhsT=va_sb[j][b0:b0 + HD,
                                              h * 128:(h + 1) * 128],
                                rhs=e_tiles[(s, hp, j)][b0:b0 + HD,
                                                        n * 512:(n + 1) * 512],
                                start=(j == 0), stop=(j == MT - 1))
                    if j == MT - 1:
                        i0 = half * 1024 + n * 512
                        cmbs, recs = [], []
                        for hp in range(2):
                            lo = bstate.pop((s, n, hp, 0))
                            hi = bstate.pop((s, n, hp, 1))
                            # DVE cannot read two PSUM operands in one op:
                            # stage lo in SBUF, then add hi (PSUM) to it.
                            los = cp.tile([128, 512], f32, name="los",
                                          tag="los")
                            nc.vector.tensor_copy(los[:], lo[:])
                            cmb = cp.tile([128, 512], f32, name="cmb",
                                          tag="cmb")
                            nc.vector.tensor_tensor(
                                out=cmb[:], in0=hi[:], in1=los[:],
                                op=mybir.AluOpType.add)
                            rec = sp.tile([1, 512], f32, name="rec", tag="rec")
                            nc.vector.reciprocal_approx_fast(
                                out=rec[:], in_=cmb[0:1, :])
                            recb = dp.tile([128, 512], f32,
                                           name="recb", tag="recb")
                            nc.gpsimd.partition_broadcast(recb[:], rec[:])
                            cmbs.append(cmb)
                            recs.append(recb)
                        for hp in range(2):
                            nc.vector.tensor_tensor(
                                out=ot_sb[pair][half][hp * HD:hp * HD + HD,
                                                      n * 512:(n + 1) * 512],
                                in0=cmbs[hp][HD:128, :],
                                in1=recs[hp][HD:128, :],
                                op=mybir.AluOpType.mult)

                def emit_vproj(st):
                    # 64-contraction lo/hi halves on T0/T8 so the A_0 window
                    # stays in one PE tiling mode; combined during bias add.
                    pv_lo = ps1("T6")
                    pv_hi = ps1("T7")
                    for k in range(KT):
                        for b0, pv in ((0, pv_lo), (HD, pv_hi)):
                            nc.tensor.matmul(
                                pv[:, 0:F],
                                lhsT=xv_sb[st][b0:b0 + HD,
                                               k * 128:(k + 1) * 128],
                                rhs=wv_sb[k][b0:b0 + HD, :],
                                start=(k == 0), stop=(k == KT - 1))
                    lvs = cp.tile([128, F], f32, name="lvs", tag="lvs")
                    nc.vector.tensor_copy(lvs[:], pv_lo[:, 0:F])
                    cv = cp.tile([128, F], f32, name="cmbv", tag="cmbv")
                    nc.vector.tensor_tensor(
                        out=cv[:], in0=pv_hi[:, 0:F], in1=lvs[:],
                        op=mybir.AluOpType.add)
                    for h in range(GH):
                        nc.vector.tensor_tensor(
                            out=va_sb[st][:, h * 128 + HD:(h + 1) * 128],
                            in0=cv[:, h * HD:(h + 1) * HD],
                            in1=bvb[:, h * HD:(h + 1) * HD],
                            op=mybir.AluOpType.add)

                def outproj_job(ih, do, n, acc, use_scalar):
                    # one [128,512] i-chunk of one do-tile; bias adds
                    # alternate VectorE/ScalarE across jobs
                    i0 = ih * 1024 + n * 512
                    for t in range(FT):
                        nc.tensor.matmul(
                            acc,
                            lhsT=wo_sb[t][:, do * 128:(do + 1) * 128],
                            rhs=ot_sb[t][ih][:, n * 512:(n + 1) * 512],
                            start=(t == 0), stop=(t == FT - 1))
                    stg = osp.tile([128, 512], bf16, name="stg", tag="stg")
                    if use_scalar:
                        nc.scalar.add(stg[:], acc, bot[:, do:do + 1])
                    else:
                        nc.vector.tensor_scalar_add(
                            stg[:], acc, bot[:, do:do + 1])
                    for i in range(2):
                        nc.sync.dma_start(
                            out=out_d[do * 128 + i * 64:
                                      do * 128 + (i + 1) * 64,
                                      i0:i0 + 512],
                            in_=stg[i * 64:(i + 1) * 64, :])

                def slot_gen(extra):
                    # lazy PSUM slot rotation: SA/SB halves (+T4..T7 once
                    # the AV accumulators have drained)
                    while True:
                        tA = ps2("SA")
                        yield tA[:, 0:512]
                        yield tA[:, 512:1024]
                        tB = ps2("SB")
                        yield tB[:, 0:512]
                        yield tB[:, 512:1024]
                        if extra:
                            for t in ("T4", "T5", "T6", "T7"):
                                yield ps1(t)[:, :]

                def run_outproj(jobs, slots):
                    for ji, (ih, do, n) in enumerate(jobs):
                        outproj_job(ih, do, n, next(slots), ji % 2 == 1)

                NW = 2 * MT

                # A_0 window: scores for seg 0, vproj fill on odd steps,
                # and the remaining qproj groups as extra PE filler
                for w in range(NW):
                    if w % 2 == 0:
                        emit_scores(0, w // 2)
                    elif w // 2 < MT:
                        emit_vproj(w // 2)
                    if w in (2, 8, 14):
                        sub = qrest[(w - 2) // 6]
                        qproj_pair(sub, [ps1("T4")[:, :], ps1("T5")[:, :]])

                # [A_s+1 || B_s] windows; at window start the scores go
                # first (nothing pending on them) so ScalarE restarts early
                for s in range(len(SEGS) - 1):
                    for w in range(NW):
                        if w == 0:
                            emit_scores(s + 1, 0)
                            emit_av(s, w)
                        else:
                            emit_av(s, w)
                            if w % 2 == 0:
                                emit_scores(s + 1, w // 2)

                # B_3 window: i-half-0 jobs interleave straight into the AV
                # n0 block (no dependency on B_3), h1-n0 jobs into the n1
                # block (they only need n0's divides); only the final 8
                # h1-n1 jobs trail the last divides, on independent 1-bank
                # slots for maximum pipeline depth.
                s3 = len(SEGS) - 1
                h0jobs = [(0, do, n) for do in range(DT) for n in range(2)]
                slots = slot_gen(False)
                for w in range(MT):
                    emit_av(s3, w)
                    for _ in range(2):
                        if h0jobs:
                            ih, do, n = h0jobs.pop(0)
                            outproj_job(ih, do, n, next(slots), do % 2 == 1)
                h1n0 = [(1, do, 0) for do in range(DT)]
                for w in range(MT, NW):
                    emit_av(s3, w)
                    if h1n0:
                        ih, do, n = h1n0.pop(0)
                        outproj_job(ih, do, n, next(slots), do % 2 == 1)

                def t_first_slots():
                    while True:
                        for t in ("T4", "T5", "T6", "T7"):
                            yield ps1(t)[:, :]
                        tA = ps2("SA")
                        yield tA[:, 0:512]
                        yield tA[:, 512:1024]
                        tB = ps2("SB")
                        yield tB[:, 0:512]
                        yield tB[:, 512:1024]

                run_outproj([(1, do, 1) for do in range(DT)], t_first_slots())

    nc.compile()
    return nc


def kernel(query, key, value, src_mask, Wq, bq, Wk, bk, Wv, bv, Wo, bo, nhead):
    global LAST_EXEC_NS, LAST_RESULTS
    import ml_dtypes
    from concourse.bass_utils import run_bass_kernel_spmd

    assert int(nhead) == H
    bf16 = ml_dtypes.bfloat16
    query = np.asarray(query, dtype=np.float32)
    key = np.asarray(key, dtype=np.float32)
    value = np.asarray(value, dtype=np.float32)
    src_mask = np.asarray(src_mask)
    Wq, bq = np.asarray(Wq, np.float32), np.asarray(bq, np.float32)
    Wk, bk = np.asarray(Wk, np.float32), np.asarray(bk, np.float32)
    Wv, bv = np.asarray(Wv, np.float32), np.asarray(bv, np.float32)
    Wo, bo = np.asarray(Wo, np.float32), np.asarray(bo, np.float32)

    # host-side key/value compaction
    idxs = [np.nonzero(~src_mask[b])[0] for b in range(B)]
    Ms = [len(ix) for ix in idxs]
    MT = max(2, (max(Ms) + 127) // 128)
    MP = MT * 128

    if ("nc", MT) not in _STATE:
        _STATE[("nc", MT)] = _build(MT)
    nc = _STATE[("nc", MT)]

    xqT, xkT, xvT, maskf = [], [], [], []
    for b in range(B):
        xqT.append(np.ascontiguousarray(query[b].T).astype(bf16))
        kc = np.zeros((MP, D), np.float32)
        kc[:Ms[b]] = key[b][idxs[b]]
        vc = np.zeros((MP, D), np.float32)
        vc[:Ms[b]] = value[b][idxs[b]]
        xkT.append(np.ascontiguousarray(kc.T).astype(bf16))
        xvT.append(np.ascontiguousarray(
            vc.T.reshape(KT, 128, MT, 128).transpose(2, 1, 0, 3)
            .reshape(MT, 128, D)).astype(bf16))
        mk = np.where(np.arange(MP) < Ms[b], np.float32(0), NEG)
        maskf.append(np.ascontiguousarray(mk.reshape(MT, 128).T))

    wqT, wkT, wvT, woT, bqs, bks, bvs = [], [], [], [], [], [], []
    for g in range(NCORES // B):
        gs, ge = g * F, (g + 1) * F
        wqT.append(np.ascontiguousarray(Wq[gs:ge, :].T).astype(bf16))
        wkT.append(np.ascontiguousarray(Wk[gs:ge, :].T).astype(bf16))
        wvT.append(np.ascontiguousarray(Wv[gs:ge, :].T).astype(bf16))
        woT.append(np.ascontiguousarray(Wo[:, gs:ge].T).astype(bf16))
        bqs.append(np.ascontiguousarray(bq[gs:ge].reshape(FT, 128).T))
        bks.append(np.ascontiguousarray(bk[gs:ge].reshape(FT, 128).T))
        bvs.append(np.ascontiguousarray(
            np.tile(bv[gs:ge][None, :], (128, 1)).astype(np.float32)))
    bo2 = np.ascontiguousarray(bo.reshape(DT, 128).T)
    bo_zero = np.zeros_like(bo2)

    in_maps = []
    for c in range(NCORES):
        b, g = c // (NCORES // B), c % (NCORES // B)
        in_maps.append({
            "xqT": xqT[b], "xkT": xkT[b], "xv3": xvT[b],
            "wqT": wqT[g], "wkT": wkT[g], "wvT": wvT[g], "woT": woT[g],
            "bq2": bqs[g], "bk2": bks[g], "bvb": bvs[g],
            "bo2": bo2 if g == 0 else bo_zero,
            "mask2": maskf[b],
        })

    kwargs = {}
    if TRACE:
        kwargs = dict(trace=True)
    res = run_bass_kernel_spmd(nc, in_maps, core_ids=list(range(NCORES)),
                               **kwargs)
    LAST_EXEC_NS = res.exec_time_ns
    LAST_RESULTS = res

    out = np.empty((B, S, D), dtype=np.float32)
    for b in range(B):
        acc = res.results[b * (NCORES // B)]["outT"].astype(np.float32)
        for g in range(1, NCORES // B):
            acc = acc + res.results[b * (NCORES // B) + g]["outT"]
        out[b] = acc.T
    return out
